# revision 58
# baseline (speedup 1.0000x reference)
"""AgentCollisionLoss Trainium2 kernel.

Full inputs -> full output. Shards the N (sample) dim across 8 NeuronCores
(2 samples per core), computes the pairwise agent-collision loss on device,
and gathers the per-core (NL, B) losses into the full (B, N) output.

Device layout (per core):
  partition p = n_local*T + t            (104 rows)
  Stage A: world-frame disk centroids CXY [P, 2*B*D] from x + per-agent consts
  Stage B: per scene block, outer-difference over the packed disk-point list,
           squares on ACT, add + two-stage min-reduce over (dj, di) on DVE
  Stage C: sqrt, penalty = relu(1 - dist/pd) on the packed pair list
  Stage D: time-decay-weighted sum over t via a [P,2]^T @ [P,32] matmul,
           moving-mask, DMA out [2, 32]

All broadcast constants ride in the packed input tensors (replicated per
partition on the host): xina carries the stage-A-critical columns so compute
starts as soon as it lands; xinb carries the stage-C constants.
"""

import os
import sys

import numpy as np

for _p in ("/opt/trn_rl_repo", "/root/.axon_site/_ro/trn_rl_repo"):
    if os.path.isdir(_p) and _p not in sys.path:
        sys.path.insert(0, _p)

import bass_rust
import concourse.bass as bass
import concourse.mybir as mybir
import concourse.tile as tile
from concourse.bass_utils import run_bass_kernel_spmd
from concourse.vector_clock import ScopedClock


def _split_drain_and_barrier(self, tick_clock, wait_clock):
    """Kernel-tail drain, one semaphore per drain instruction.

    The walrus build in this container rejects instructions carrying more
    than one embedded sync wait ("Too many sync wait commands"). Tile's
    stock tail emits a single drain waiting on the full global clock, so
    split it: one drain per nonzero proc tick. add_sem_waits elides waits
    the engine has already observed, so each drain carries exactly one.
    """
    gc = list(tick_clock.global_clock)
    engs = [self.nc.sync, self.nc.vector, self.nc.scalar, self.nc.gpsimd,
            self.nc.tensor]
    nd = 0
    for idx, tick in enumerate(gc):
        if tick <= 0:
            continue
        v = [0] * len(gc)
        v[idx] = tick
        d = engs[nd % len(engs)].drain()
        nd += 1
        wait_clock.add_sem_waits(
            d.ins, ScopedClock({None: bass_rust.VectorClock(v)})
        )
    self.nc.all_engine_barrier()
    assert self.sems is not None
    popped = self.nc._tile_sem_poison_stack.pop()
    assert popped is self._sem_poison
    self.nc.clear_and_free_semaphores(list(self.sems.allocated().values()))
    self.nc.all_engine_barrier()


tile.TileContext._drain_and_barrier = _split_drain_and_barrier

B, N, T, D = 32, 16, 52, 5
NCORES = 8
NL = N // NCORES          # samples per core
P = NL * T                # partition rows per core
BUFFER_DIST = 0.2
DECAY_RATE = 0.9
SPEED_TH = 0.5
FMAX = 4000               # max free elems per big-stage chunk

F32 = mybir.dt.float32
F16 = mybir.dt.float16
PI = float(np.pi)

# bulk dtype for squared distances (precision analysed: d2 < 43000 < f16 max,
# and only d2 <= pd^2 ~ 16 matters, where f16 ulp ~ 0.008-0.016)
DT_BULK = F16
# gpsimd measured rates: 2-input fp16 add ~2.05 ns/elem, but broadcast-AP
# f32 subs ~3-3.6 ns/elem (worse than leaving them on DVE). So only the
# d2-adds of the non-largest chunks go to gpsimd.
SUB_ON_GPSIMD = ()
GPSIMD_ADD_MAX_F = 950


def _rects(scenes):
    """Circulant half-pair rects per scene, largest first.

    Each unordered same-scene pair {i, j} is covered exactly once:
    rect A: (i, k) for i in [0,s), k in [1,K], j = (i+k) mod s, K=(s-1)//2
    rect B (even s): (i, s/2) for i in [0, s/2), j = i + s/2
    Returns [(o, s, K, half)] with half = s//2 if s even else 0.
    """
    out = []
    for (o, s) in scenes:
        K = (s - 1) // 2
        half = s // 2 if s % 2 == 0 else 0
        out.append((o, s, K, half))
    out.sort(key=lambda r: -(r[1] * r[2] + r[3]))
    return out


# xinA column layout (stage-A-critical): x(3B) | geo(8B)
XO_GEO = 3 * B
XWA = XO_GEO + 8 * B
# xinB column layout: cent(B*D) | mvr(NL*B) | prc(PP) | wmt(NL)
XO_CENT = 0
XO_MVR = XO_CENT + B * D
XO_PRC = XO_MVR + NL * B


def _xin_width_b(PP):
    return XO_PRC + PP + NL


def _build_nc(scenes, PP):
    """Build the SPMD Bass program. `scenes` = [(offset, size)], PP = sum s^2."""
    nc = bass.Bass()

    XWB = _xin_width_b(PP)
    xina = nc.dram_tensor("xina", [P, XWA], F32, kind="ExternalInput")
    xinb = nc.dram_tensor("xinb", [P, XWB], F32, kind="ExternalInput")
    mmat = nc.dram_tensor("mmat", [128, B], F16, kind="ExternalInput")
    wmv = nc.dram_tensor("wmv", [B, P + 1], F32, kind="ExternalInput")
    out = nc.dram_tensor("loss", [B, NL], F32, kind="ExternalOutput")

    rects = _rects(scenes)

    with tile.TileContext(nc) as tc:
        with (
            tc.tile_pool(name="singles", bufs=1) as singles,
            tc.tile_pool(name="small", bufs=1) as small,
            tc.tile_pool(name="big", bufs=1) as big,
            tc.tile_pool(name="psum", bufs=1, space="PSUM") as psum,
        ):
            # ---- loads (stage-A-critical part first) ----
            xta = singles.tile([P, XWA], F32)
            nc.sync.dma_start(out=xta[:], in_=xina[:])
            xtb = singles.tile([P, XWB], F32)
            nc.sync.dma_start(out=xtb[:], in_=xinb[:])
            ones = singles.tile([P, 1], F32)
            nc.vector.memset(ones[:], 1.0)

            # Pre-touch the DMA'd tiles on DVE: each copy carries one
            # DMA-queue sem wait, so later compute ops joining DMA data with
            # engine-produced data need at most one new wait (this walrus
            # rejects instructions with more than one embedded sync wait).
            tch = singles.tile([P, 1], F32, tag="tch0")
            nc.vector.tensor_copy(out=tch[:], in_=xta[:, 0:1])
            tchb = singles.tile([P, 1], F32, tag="tchb")
            nc.vector.tensor_copy(out=tchb[:], in_=xtb[:, 0:1])
            # pair-endpoint incidence matrix (matmul weights) and w/moving
            mt = singles.tile([128, B], F16)
            nc.sync.dma_start(out=mt[:], in_=mmat[:])
            wmvt = singles.tile([B, P + 1], F32)
            nc.sync.dma_start(out=wmvt[:], in_=wmv[:])
            tchw = singles.tile([B, 1], F32, tag="tchw")
            nc.vector.tensor_copy(out=tchw[:], in_=wmvt[:, 0:1])
            # route the weights through DVE so load_weights' dep is DVE-only
            mt2 = singles.tile([128, B], F16)
            nc.vector.tensor_copy(out=mt2[:], in_=mt[:])

            gA = xta[:, XO_GEO + 0 * B : XO_GEO + 2 * B]
            gB = xta[:, XO_GEO + 2 * B : XO_GEO + 4 * B]
            gT = xta[:, XO_GEO + 4 * B : XO_GEO + 6 * B]
            shifts2 = xta[:, XO_GEO + 6 * B : XO_GEO + 8 * B]
            x0 = xta[:, 0:B]
            x1 = xta[:, B : 2 * B]
            yw = xta[:, 2 * B : 3 * B]
            cxc = xtb[:, XO_CENT : XO_CENT + B * D]
            movt = xtb[0:NL, XO_MVR : XO_MVR + B]   # replicated const rows
            prc = xtb[:, XO_PRC : XO_PRC + PP]

            def rep2(apx, w):
                """view [P, 2, w] reading apx's first w elems twice"""
                return bass.AP(tensor=apx.tensor, offset=apx.offset,
                               ap=[apx.ap[0], [0, 2], [1, w]])

            # ---- stage A ----
            # u = yaw/2pi + (shift + yoff/2pi)   (shift 2.0 -> sin, 2.25 -> cos)
            u2 = small.tile([P, 2, B], F32)
            nc.vector.scalar_tensor_tensor(
                out=u2[:], in0=rep2(yw, B), scalar=1.0 / (2.0 * PI),
                in1=shifts2.rearrange("p (c i) -> p c i", c=2),
                op0=mybir.AluOpType.mult, op1=mybir.AluOpType.add)
            # round-to-nearest-even via the 1.5*2^23 magic constant
            MAGIC = 12582912.0
            kf = small.tile([P, 2, B], F32)
            nc.vector.tensor_scalar(
                out=kf[:], in0=u2[:], scalar1=MAGIC, scalar2=MAGIC,
                op0=mybir.AluOpType.add, op1=mybir.AluOpType.subtract)
            fr = small.tile([P, 2, B], F32)
            nc.vector.tensor_sub(fr[:], u2[:], kf[:])
            # sincos[:, 0:32] = sin(yawg), [:, 32:64] = cos(yawg)
            sincos = small.tile([P, 2 * B], F32)
            nc.scalar.activation(out=sincos[:].rearrange("p (c i) -> p c i", c=2),
                                 in_=fr[:],
                                 func=mybir.ActivationFunctionType.Sin,
                                 bias=0.0, scale=2.0 * PI)

            # pos_g for both coords: pg[p, c, i], c=0 -> x, 1 -> y
            # m12[p, xsel, c, i] = x_xsel * g_{xsel,c}  in one multiply
            m12 = small.tile([P, 2, 2, B], F32)
            xx = bass.AP(tensor=xta.tensor, offset=x0.offset,
                         ap=[x0.ap[0], [B, 2], [0, 2], [1, B]])
            gAB = bass.AP(tensor=xta.tensor, offset=gA.offset,
                          ap=[gA.ap[0], [2 * B, 2], [B, 2], [1, B]])
            nc.vector.tensor_mul(m12[:], xx, gAB)
            pg = small.tile([P, 2, B], F32)
            nc.vector.tensor_add(pg[:], m12[:, 0], m12[:, 1])
            nc.vector.tensor_add(pg[:], pg[:],
                                 gT.rearrange("p (c i) -> p c i", c=2))

            # CXY[p, c, i, di] = cent_x(i,di) * cs(c,i) + pg(c,i)
            # c=0 uses cos, c=1 uses sin (x = cx*cos + pgx, y = cx*sin + pgy)
            cxy = singles.tile([P, 2, B, D], F32)
            cs_sel = bass.AP(tensor=sincos.tensor, offset=sincos[:].offset + B,
                             ap=[sincos[:].ap[0], [-B, 2], [1, B], [0, D]])
            cx_rep = bass.AP(tensor=xtb.tensor, offset=cxc.offset,
                             ap=[cxc.ap[0], [0, 2], [D, B], [1, D]])
            pg_bc = bass.AP(tensor=pg.tensor, offset=pg[:].offset,
                            ap=[pg[:].ap[0], [B, 2], [1, B], [0, D]])
            nc.vector.tensor_mul(cxy[:], cx_rep, cs_sel)
            nc.vector.tensor_add(cxy[:], cxy[:], pg_bc)

            cxyf = cxy[:].rearrange("p c i d -> p (c i d)")
            pap = cxyf.ap[0]
            e = cxyf.ap[-1][0]

            # ---- stage B: circulant half-pair rects ----
            pdist = singles.tile([P, PP], F32)
            NPTS = B * D

            # doubled per-scene point lists so the wrap in j = (i+k) mod s
            # becomes a plain linear read: cxy2 block for scene (o,s) holds
            # its 5s points twice, per coord
            DBL = 2 * NPTS
            cxy2 = singles.tile([P, 2, DBL], F32)
            c2f = cxy2[:].rearrange("p c d -> p (c d)")
            pap2 = c2f.ap[0]
            e2 = c2f.ap[-1][0]
            dbl_off = {}
            do_ = 0
            for (o, s) in scenes:
                dbl_off[o] = do_
                in_ap = bass.AP(tensor=cxyf.tensor,
                                offset=cxyf.offset + o * D * e,
                                ap=[pap, [NPTS * e, 2], [0, 2], [e, D * s]])
                out_ap = bass.AP(tensor=c2f.tensor,
                                 offset=c2f.offset + do_ * e2,
                                 ap=[pap2, [DBL * e2, 2], [D * s * e2, 2],
                                     [e2, D * s]])
                nc.vector.tensor_copy(out=out_ap, in_=in_ap)
                do_ += 2 * D * s

            poffs = []
            po = 0
            for (o, s, K, half) in rects:
                poffs.append(po)
                po += s * K + half
            assert po == PP

            subsA, subsB = {}, {}
            for idx, (o, s, K, half) in enumerate(rects):
                m, w = D * s, D * K
                if K >= 1:
                    sub = big.tile([P, 2, s, D, w], DT_BULK, tag=f"sA{idx}")
                    for c in range(2):
                        a_ap = bass.AP(
                            tensor=cxyf.tensor,
                            offset=cxyf.offset + (c * NPTS + o * D) * e,
                            ap=[pap, [D * e, s], [e, D], [0, w]])
                        b_ap = bass.AP(
                            tensor=c2f.tensor,
                            offset=c2f.offset + (c * DBL + dbl_off[o] + D) * e2,
                            ap=[pap2, [D * e2, s], [0, D], [e2, w]])
                        nc.vector.tensor_tensor(out=sub[:, c], in0=a_ap,
                                                in1=b_ap,
                                                op=mybir.AluOpType.subtract)
                    subsA[idx] = sub
                if half:
                    subh = big.tile([P, 2, half, D, D], DT_BULK, tag=f"sB{idx}")
                    for c in range(2):
                        a_ap = bass.AP(
                            tensor=cxyf.tensor,
                            offset=cxyf.offset + (c * NPTS + o * D) * e,
                            ap=[pap, [D * e, half], [e, D], [0, D]])
                        b_ap = bass.AP(
                            tensor=cxyf.tensor,
                            offset=cxyf.offset + (c * NPTS + (o + half) * D) * e,
                            ap=[pap, [D * e, half], [0, D], [e, D]])
                        nc.vector.tensor_tensor(out=subh[:, c], in0=a_ap,
                                                in1=b_ap,
                                                op=mybir.AluOpType.subtract)
                    subsB[idx] = subh

            def square_pair(sub, F, tagp):
                sq = {}
                subf = sub[:].rearrange("p c a b q -> p (c a b q)")
                es = subf.ap[-1][0]
                for c, nm in ((0, "x"), (1, "y")):
                    tsq = big.tile([P, F], DT_BULK, tag=f"{tagp}{nm}")
                    src_ap = bass.AP(tensor=subf.tensor,
                                     offset=subf.offset + c * F * es,
                                     ap=[subf.ap[0], [es, F]])
                    nc.scalar.activation(
                        out=tsq[:], in_=src_ap,
                        func=mybir.ActivationFunctionType.Square)
                    sq[nm] = tsq
                return sq

            sqsA, sqsB = {}, {}
            for idx, (o, s, K, half) in enumerate(rects):
                if K >= 1:
                    sqsA[idx] = square_pair(subsA[idx], s * D * D * K, f"qA{idx}")
                if half:
                    sqsB[idx] = square_pair(subsB[idx], half * D * D, f"qB{idx}")

            for idx, (o, s, K, half) in enumerate(rects):
                poff = poffs[idx]
                m, w = D * s, D * K
                if K >= 1:
                    F = s * D * w
                    d2 = big.tile([P, F], DT_BULK, tag=f"dA{idx}")
                    add_eng = nc.gpsimd if F <= GPSIMD_ADD_MAX_F else nc.vector
                    add_eng.tensor_tensor(out=d2[:], in0=sqsA[idx]["x"][:],
                                          in1=sqsA[idx]["y"][:],
                                          op=mybir.AluOpType.add)
                    # min over dj; scatter-write r1 in (i, k, di) order
                    ed = d2[:].ap[-1][0]
                    d2v = bass.AP(tensor=d2.tensor, offset=d2[:].offset,
                                  ap=[d2[:].ap[0], [w * ed, m],
                                      [D * ed, K], [ed, D]])
                    r1 = big.tile([P, s, K, D], DT_BULK, tag=f"rA{idx}")
                    r1f = r1[:].rearrange("p a b c -> p (a b c)")
                    e1 = r1f.ap[-1][0]
                    r1scat = bass.AP(tensor=r1f.tensor, offset=r1f.offset,
                                     ap=[r1f.ap[0], [K * D * e1, s],
                                         [e1, D], [D * e1, K]])
                    nc.vector.tensor_reduce(out=r1scat, in_=d2v,
                                            axis=mybir.AxisListType.X,
                                            op=mybir.AluOpType.min)
                    pmin = pdist[:, poff : poff + s * K].rearrange(
                        "p (a b) -> p a b", b=K)
                    nc.vector.tensor_reduce(out=pmin, in_=r1[:],
                                            axis=mybir.AxisListType.X,
                                            op=mybir.AluOpType.min)
                if half:
                    Fh = half * D * D
                    dh = big.tile([P, Fh], DT_BULK, tag=f"dB{idx}")
                    nc.vector.tensor_tensor(out=dh[:], in0=sqsB[idx]["x"][:],
                                            in1=sqsB[idx]["y"][:],
                                            op=mybir.AluOpType.add)
                    r1h = big.tile([P, half, D], DT_BULK, tag=f"rB{idx}")
                    nc.vector.tensor_reduce(
                        out=r1h[:],
                        in_=dh[:].rearrange("p (a dj) -> p a dj", dj=D),
                        axis=mybir.AxisListType.X, op=mybir.AluOpType.min)
                    ph = pdist[:, poff + s * K : poff + s * K + half]
                    nc.vector.tensor_reduce(out=ph, in_=r1h[:],
                                            axis=mybir.AxisListType.X,
                                            op=mybir.AluOpType.min)

            # ---- stage C: penalties on the packed half-pair list ----
            dist = small.tile([P, PP], F32, tag="dist")
            nc.scalar.activation(out=dist[:], in_=pdist[:],
                                 func=mybir.ActivationFunctionType.Sqrt)
            rr = small.tile([P, PP], F32, tag="rr")
            nc.vector.tensor_mul(rr[:], dist[:], prc)
            # [112, 128]: DMA transpose needs src partitions % 16 == 0 and
            # src free % 128 == 0; the padding is zeroed (zero rows of the
            # transposed result contribute nothing through the matmul)
            pen = small.tile([112, 128], DT_BULK, tag="pen")
            nc.vector.memset(pen[:], 0.0)
            nc.scalar.activation(out=pen[0:P, 0:PP], in_=rr[:],
                                 func=mybir.ActivationFunctionType.Relu,
                                 bias=ones[:], scale=-1.0)

            # ---- reduction: transpose pen once, then one PE matmul with
            # the 0/1 pair-endpoint incidence matrix does every row+partner
            # sum; the time-decay sum becomes a free-dim reduce ----
            penT = singles.tile([128, 112], DT_BULK)
            # two 1-elem observer DMAs let the sync queue see pen's two
            # writer semaphores (DVE memset, ACT relu) one at a time, so the
            # transpose itself carries only the xbar-transition wait
            scrA = nc.dram_tensor("scrA", [1, 1], DT_BULK)
            scrB = nc.dram_tensor("scrB", [1, 1], DT_BULK)
            nc.sync.dma_start(out=scrA[:], in_=pen[104:105, 0:1])
            nc.sync.dma_start(out=scrB[:], in_=pen[0:1, 0:1])
            nc.sync.dma_start(out=penT[:], in_=pen[:], transpose=True)
            ps = psum.tile([B, 112], F32)
            nc.tensor.ldweights(mt2[:])
            nc.tensor.matmul(ps[:], mt2[:], penT[:], start=True, stop=True)
            # s32 = lossT * w  (w includes 1/B), then sum over t
            s32 = small.tile([B, P], F32, tag="s32")
            nc.vector.tensor_mul(s32[:], ps[:, 0:P], wmvt[:, 0:P])
            ln = small.tile([B, NL], F32, tag="ln")
            nc.vector.tensor_reduce(
                out=ln[:], in_=s32[:].rearrange("b (n t) -> b n t", n=NL),
                axis=mybir.AxisListType.X, op=mybir.AluOpType.add)
            lout = small.tile([B, NL], F32, tag="lout")
            nc.vector.tensor_scalar(
                out=lout[:], in0=ln[:], scalar1=wmvt[:, P : P + 1], scalar2=None,
                op0=mybir.AluOpType.mult)
            # SWDGE for the output store; observer DMA first so the pool
            # queue sees the transpose's xbar-serialization sem separately
            scrC = nc.dram_tensor("scrC", [1, 1], DT_BULK)
            nc.gpsimd.dma_start(out=scrC[:], in_=penT[0:1, 0:1])
            nc.gpsimd.dma_start(out=out[:], in_=lout[:])

    return nc


def _prepare(inputs):
    x = np.ascontiguousarray(inputs["x"], dtype=np.float32)
    extent = np.asarray(inputs["extent"], dtype=np.float32)
    wfa = np.asarray(inputs["world_from_agent"], dtype=np.float32)
    speed = np.asarray(inputs["curr_speed"], dtype=np.float32)
    scene = np.asarray(inputs["scene_index"])

    R = wfa[:, :2, :2]
    tr = wfa[:, :2, 2]
    yaw_off = np.arctan2(R[:, 1, 0], R[:, 0, 0]).astype(np.float32)
    agt_rad = extent[:, 1] / 2.0
    cent_min = -(extent[:, 0] / 2.0) + agt_rad
    cent_max = (extent[:, 0] / 2.0) - agt_rad
    lin = np.linspace(0.0, 1.0, D, dtype=np.float32)
    cent_x = (cent_min[:, None] + (cent_max - cent_min)[:, None] * lin).astype(
        np.float32)
    pd = (agt_rad[:, None] + agt_rad[None, :] + BUFFER_DIST).astype(np.float32)
    moving = (np.abs(speed) > SPEED_TH)

    # contiguous scene blocks (scene_index is sorted)
    _, starts, counts = np.unique(scene, return_index=True, return_counts=True)
    scenes = [(int(o), int(s)) for o, s in zip(starts, counts)]
    assert sum(s for _, s in scenes) == B
    for o, s in scenes:
        assert (scene[o : o + s] == scene[o]).all()

    pairs_i = []
    pairs_j = []
    for (o, s, K, half) in _rects(scenes):
        for i in range(s):
            for k in range(1, K + 1):
                pairs_i.append(o + i)
                pairs_j.append(o + (i + k) % s)
        for i in range(half):
            pairs_i.append(o + i)
            pairs_j.append(o + i + half)
    pairs_i = np.array(pairs_i)
    pairs_j = np.array(pairs_j)
    PP = len(pairs_i)
    inv_pd = (1.0 / pd[pairs_i, pairs_j]).astype(np.float32)

    twopi = 2.0 * np.pi
    geo = np.concatenate([
        R[:, 0, 0], R[:, 1, 0],          # gA
        R[:, 0, 1], R[:, 1, 1],          # gB
        tr[:, 0], tr[:, 1],              # gT
        2.0 + yaw_off / twopi, 2.25 + yaw_off / twopi,  # shifts2
    ]).astype(np.float32)

    w = DECAY_RATE ** np.arange(T, dtype=np.float32)
    w = w / w.sum()
    wmt = np.zeros((P, NL), dtype=np.float32)
    for nl in range(NL):
        wmt[nl * T : (nl + 1) * T, nl] = w / B

    # packed inputs: per-partition x data + replicated consts + wmt
    XWB = _xin_width_b(PP)
    mvr2 = np.tile(moving.astype(np.float32), NL)
    constA = geo
    xinb_row = np.empty((P, XWB), dtype=np.float32)
    xinb_row[:, XO_CENT : XO_PRC + PP] = np.concatenate(
        [cent_x.reshape(-1), mvr2, inv_pd])[None, :]
    xinb_row[:, XO_PRC + PP :] = wmt
    # pair-endpoint incidence matrix and (w, moving) rows
    mmat_ = np.zeros((128, B), dtype=np.float16)
    for q in range(PP):
        mmat_[q, pairs_i[q]] = 1.0
        mmat_[q, pairs_j[q]] = 1.0
    wmv_ = np.empty((B, P + 1), dtype=np.float32)
    wmv_[:, :P] = np.tile(w / B, NL)[None, :]
    wmv_[:, P] = moving.astype(np.float32)
    in_maps = []
    for c in range(NCORES):
        xs = x[:, c * NL : (c + 1) * NL, :, :]          # (B, NL, T, 6)
        xs = xs[..., [0, 1, 3]]                          # (B, NL, T, 3)
        xdat = xs.transpose(1, 2, 3, 0).reshape(P, 3 * B)
        xina = np.empty((P, XWA), dtype=np.float32)
        xina[:, 0 : 3 * B] = xdat
        xina[:, XO_GEO:] = constA[None, :]
        in_maps.append({"xina": xina, "xinb": xinb_row,
                        "mmat": mmat_, "wmv": wmv_})

    return scenes, PP, in_maps, moving


_CACHE = {}


def _get_nc(scenes, PP):
    key = (tuple(scenes), PP)
    if key not in _CACHE:
        _CACHE[key] = _build_nc(scenes, PP)
    return _CACHE[key]


def _run(inputs, trace=False):
    scenes, PP, in_maps, moving = _prepare(inputs)
    nc = _get_nc(scenes, PP)
    res = run_bass_kernel_spmd(nc, in_maps, core_ids=list(range(NCORES)),
                               trace=trace)
    outf = np.zeros((B, N), dtype=np.float32)
    for c in range(NCORES):
        lc = res.results[c]["loss"]                      # (B, NL)
        for nl in range(NL):
            outf[:, c * NL + nl] = lc[:, nl]
    return outf, res


def kernel(**inputs):
    outf, _ = _run(inputs, trace=False)
    return outf


def _ensure_ntff_hook():
    """Register the axon NTFF profile hook if the container's antenv lacks it."""
    try:
        from antenv.axon_hooks import get_axon_ntff_profile_hook  # noqa: F401
        return
    except ImportError:
        pass
    import types

    if "/root/.axon_site" not in sys.path:
        sys.path.insert(0, "/root/.axon_site")
    from trn_agent_boot.trn_boot import _ntff_profile_via_ctypes

    hook = _ntff_profile_via_ctypes("/opt/axon/libaxon_pjrt.so")
    mod = types.ModuleType("antenv.axon_hooks")
    mod.get_axon_ntff_profile_hook = lambda: hook
    mod.set_axon_ntff_profile_hook = lambda h: None
    sys.modules["antenv.axon_hooks"] = mod


def run_traced(inputs):
    """Correctness output + profiled exec time (ns) via NTFF trace."""
    _ensure_ntff_hook()
    outf, res = _run(inputs, trace=True)
    return outf, res.exec_time_ns


# revision 59
# speedup vs baseline: 1.1362x; 1.1362x over previous
"""AgentCollisionLoss Trainium2 kernel.

Full inputs -> full output. Shards the N (sample) dim across 8 NeuronCores
(2 samples per core), computes the pairwise agent-collision loss on device,
and gathers the per-core (NL, B) losses into the full (B, N) output.

Device layout (per core):
  partition p = n_local*T + t            (104 rows)
  Stage A: world-frame disk centroids CXY [P, 2*B*D] from x + per-agent consts
  Stage B: per scene block, outer-difference over the packed disk-point list,
           squares on ACT, add + two-stage min-reduce over (dj, di) on DVE
  Stage C: sqrt, penalty = relu(1 - dist/pd) on the packed pair list
  Stage D: time-decay-weighted sum over t via a [P,2]^T @ [P,32] matmul,
           moving-mask, DMA out [2, 32]

All broadcast constants ride in the packed input tensors (replicated per
partition on the host): xina carries the stage-A-critical columns so compute
starts as soon as it lands; xinb carries the stage-C constants.
"""

import os
import sys

import numpy as np

for _p in ("/opt/trn_rl_repo", "/root/.axon_site/_ro/trn_rl_repo"):
    if os.path.isdir(_p) and _p not in sys.path:
        sys.path.insert(0, _p)

import bass_rust
import concourse.bass as bass
import concourse.mybir as mybir
import concourse.tile as tile
from concourse.bass_utils import run_bass_kernel_spmd
from concourse.vector_clock import ScopedClock


def _split_drain_and_barrier(self, tick_clock, wait_clock):
    """Kernel-tail drain, one semaphore per drain instruction.

    The walrus build in this container rejects instructions carrying more
    than one embedded sync wait ("Too many sync wait commands"). Tile's
    stock tail emits a single drain waiting on the full global clock, so
    split it: one drain per nonzero proc tick. add_sem_waits elides waits
    the engine has already observed, so each drain carries exactly one.
    """
    gc = list(tick_clock.global_clock)
    engs = [self.nc.sync, self.nc.vector, self.nc.scalar, self.nc.gpsimd,
            self.nc.tensor]
    nd = 0
    for idx, tick in enumerate(gc):
        if tick <= 0:
            continue
        v = [0] * len(gc)
        v[idx] = tick
        d = engs[nd % len(engs)].drain()
        nd += 1
        wait_clock.add_sem_waits(
            d.ins, ScopedClock({None: bass_rust.VectorClock(v)})
        )
    self.nc.all_engine_barrier()
    assert self.sems is not None
    popped = self.nc._tile_sem_poison_stack.pop()
    assert popped is self._sem_poison
    self.nc.clear_and_free_semaphores(list(self.sems.allocated().values()))
    self.nc.all_engine_barrier()


tile.TileContext._drain_and_barrier = _split_drain_and_barrier

B, N, T, D = 32, 16, 52, 5
NCORES = 8
NL = N // NCORES          # samples per core
P = NL * T                # partition rows per core
BUFFER_DIST = 0.2
DECAY_RATE = 0.9
SPEED_TH = 0.5
FMAX = 4000               # max free elems per big-stage chunk

F32 = mybir.dt.float32
F16 = mybir.dt.float16
PI = float(np.pi)

# bulk dtype for squared distances (precision analysed: d2 < 43000 < f16 max,
# and only d2 <= pd^2 ~ 16 matters, where f16 ulp ~ 0.008-0.016)
DT_BULK = F16
# gpsimd measured rates: 2-input fp16 add ~2.05 ns/elem, but broadcast-AP
# f32 subs ~3-3.6 ns/elem (worse than leaving them on DVE). So only the
# d2-adds of the non-largest chunks go to gpsimd.
SUB_ON_GPSIMD = ()
GPSIMD_ADD_MAX_F = 950


def _rects(scenes):
    """Circulant half-pair rects per scene, largest first.

    Each unordered same-scene pair {i, j} is covered exactly once:
    rect A: (i, k) for i in [0,s), k in [1,K], j = (i+k) mod s, K=(s-1)//2
    rect B (even s): (i, s/2) for i in [0, s/2), j = i + s/2
    Returns [(o, s, K, half)] with half = s//2 if s even else 0.
    """
    out = []
    for (o, s) in scenes:
        K = (s - 1) // 2
        half = s // 2 if s % 2 == 0 else 0
        out.append((o, s, K, half))
    out.sort(key=lambda r: -(r[1] * r[2] + r[3]))
    return out


# xinA column layout (stage-A-critical): x(3B) | geo(8B)
XO_GEO = 3 * B
XWA = XO_GEO + 8 * B
# xinB column layout: cent(B*D) | mvr(NL*B) | prc(PP) | wmt(NL)
XO_CENT = 0
XO_MVR = XO_CENT + B * D
XO_PRC = XO_MVR + NL * B


def _xin_width_b(PP):
    return XO_PRC + PP + NL


def _build_nc(scenes, PP):
    """Build the SPMD Bass program. `scenes` = [(offset, size)], PP = sum s^2."""
    nc = bass.Bass()

    XWB = _xin_width_b(PP)
    xina = nc.dram_tensor("xina", [P, XWA], F32, kind="ExternalInput")
    xinb = nc.dram_tensor("xinb", [P, XWB], F32, kind="ExternalInput")
    out = nc.dram_tensor("loss", [NL, B], F32, kind="ExternalOutput")

    rects = _rects(scenes)

    with tile.TileContext(nc) as tc:
        with (
            tc.tile_pool(name="singles", bufs=1) as singles,
            tc.tile_pool(name="small", bufs=1) as small,
            tc.tile_pool(name="big", bufs=1) as big,
            tc.tile_pool(name="psum", bufs=1, space="PSUM") as psum,
        ):
            # ---- loads (stage-A-critical part first) ----
            xta = singles.tile([P, XWA], F32)
            nc.sync.dma_start(out=xta[:], in_=xina[:])
            xtb = singles.tile([P, XWB], F32)
            nc.sync.dma_start(out=xtb[:], in_=xinb[:])
            ones = singles.tile([P, 1], F32)
            nc.vector.memset(ones[:], 1.0)

            # Pre-touch the DMA'd tiles on DVE: each copy carries one
            # DMA-queue sem wait, so later compute ops joining DMA data with
            # engine-produced data need at most one new wait (this walrus
            # rejects instructions with more than one embedded sync wait).
            tch = singles.tile([P, 1], F32, tag="tch0")
            nc.vector.tensor_copy(out=tch[:], in_=xta[:, 0:1])
            tchb = singles.tile([P, 1], F32, tag="tchb")
            nc.vector.tensor_copy(out=tchb[:], in_=xtb[:, 0:1])
            # matmul weights via DVE so the PE matmul's deps are DVE-only
            wt2 = singles.tile([P, NL], F32)
            nc.vector.tensor_copy(out=wt2[:], in_=xtb[:, XO_PRC + PP : XO_PRC + PP + NL])

            gA = xta[:, XO_GEO + 0 * B : XO_GEO + 2 * B]
            gB = xta[:, XO_GEO + 2 * B : XO_GEO + 4 * B]
            gT = xta[:, XO_GEO + 4 * B : XO_GEO + 6 * B]
            shifts2 = xta[:, XO_GEO + 6 * B : XO_GEO + 8 * B]
            x0 = xta[:, 0:B]
            x1 = xta[:, B : 2 * B]
            yw = xta[:, 2 * B : 3 * B]
            cxc = xtb[:, XO_CENT : XO_CENT + B * D]
            movt = xtb[0:NL, XO_MVR : XO_MVR + B]   # replicated const rows
            prc = xtb[:, XO_PRC : XO_PRC + PP]

            def rep2(apx, w):
                """view [P, 2, w] reading apx's first w elems twice"""
                return bass.AP(tensor=apx.tensor, offset=apx.offset,
                               ap=[apx.ap[0], [0, 2], [1, w]])

            # ---- stage A ----
            # u = yaw/2pi + (shift + yoff/2pi)   (shift 2.0 -> sin, 2.25 -> cos)
            u2 = small.tile([P, 2, B], F32)
            nc.vector.scalar_tensor_tensor(
                out=u2[:], in0=rep2(yw, B), scalar=1.0 / (2.0 * PI),
                in1=shifts2.rearrange("p (c i) -> p c i", c=2),
                op0=mybir.AluOpType.mult, op1=mybir.AluOpType.add)
            # round-to-nearest-even via the 1.5*2^23 magic constant
            MAGIC = 12582912.0
            kf = small.tile([P, 2, B], F32)
            nc.vector.tensor_scalar(
                out=kf[:], in0=u2[:], scalar1=MAGIC, scalar2=MAGIC,
                op0=mybir.AluOpType.add, op1=mybir.AluOpType.subtract)
            fr = small.tile([P, 2, B], F32)
            nc.vector.tensor_sub(fr[:], u2[:], kf[:])
            # sincos[:, 0:32] = sin(yawg), [:, 32:64] = cos(yawg)
            sincos = small.tile([P, 2 * B], F32)
            nc.scalar.activation(out=sincos[:].rearrange("p (c i) -> p c i", c=2),
                                 in_=fr[:],
                                 func=mybir.ActivationFunctionType.Sin,
                                 bias=0.0, scale=2.0 * PI)

            # pos_g for both coords: pg[p, c, i], c=0 -> x, 1 -> y
            # m12[p, xsel, c, i] = x_xsel * g_{xsel,c}  in one multiply
            m12 = small.tile([P, 2, 2, B], F32)
            xx = bass.AP(tensor=xta.tensor, offset=x0.offset,
                         ap=[x0.ap[0], [B, 2], [0, 2], [1, B]])
            gAB = bass.AP(tensor=xta.tensor, offset=gA.offset,
                          ap=[gA.ap[0], [2 * B, 2], [B, 2], [1, B]])
            nc.vector.tensor_mul(m12[:], xx, gAB)
            pg = small.tile([P, 2, B], F32)
            nc.vector.tensor_add(pg[:], m12[:, 0], m12[:, 1])
            nc.vector.tensor_add(pg[:], pg[:],
                                 gT.rearrange("p (c i) -> p c i", c=2))

            # CXY[p, c, i, di] = cent_x(i,di) * cs(c,i) + pg(c,i)
            # c=0 uses cos, c=1 uses sin (x = cx*cos + pgx, y = cx*sin + pgy)
            cxy = singles.tile([P, 2, B, D], F32)
            cs_sel = bass.AP(tensor=sincos.tensor, offset=sincos[:].offset + B,
                             ap=[sincos[:].ap[0], [-B, 2], [1, B], [0, D]])
            cx_rep = bass.AP(tensor=xtb.tensor, offset=cxc.offset,
                             ap=[cxc.ap[0], [0, 2], [D, B], [1, D]])
            pg_bc = bass.AP(tensor=pg.tensor, offset=pg[:].offset,
                            ap=[pg[:].ap[0], [B, 2], [1, B], [0, D]])
            nc.vector.tensor_mul(cxy[:], cx_rep, cs_sel)
            nc.vector.tensor_add(cxy[:], cxy[:], pg_bc)

            cxyf = cxy[:].rearrange("p c i d -> p (c i d)")
            pap = cxyf.ap[0]
            e = cxyf.ap[-1][0]

            # ---- stage B: circulant half-pair rects ----
            pdist = singles.tile([P, PP], F32)
            NPTS = B * D

            # doubled per-scene point lists so the wrap in j = (i+k) mod s
            # becomes a plain linear read: cxy2 block for scene (o,s) holds
            # its 5s points twice, per coord
            DBL = 2 * NPTS
            cxy2 = singles.tile([P, 2, DBL], F32)
            c2f = cxy2[:].rearrange("p c d -> p (c d)")
            pap2 = c2f.ap[0]
            e2 = c2f.ap[-1][0]
            dbl_off = {}
            do_ = 0
            for (o, s) in scenes:
                dbl_off[o] = do_
                in_ap = bass.AP(tensor=cxyf.tensor,
                                offset=cxyf.offset + o * D * e,
                                ap=[pap, [NPTS * e, 2], [0, 2], [e, D * s]])
                out_ap = bass.AP(tensor=c2f.tensor,
                                 offset=c2f.offset + do_ * e2,
                                 ap=[pap2, [DBL * e2, 2], [D * s * e2, 2],
                                     [e2, D * s]])
                nc.vector.tensor_copy(out=out_ap, in_=in_ap)
                do_ += 2 * D * s

            poffs = []
            po = 0
            for (o, s, K, half) in rects:
                poffs.append(po)
                po += s * K + half
            assert po == PP

            subsA, subsB = {}, {}
            for idx, (o, s, K, half) in enumerate(rects):
                m, w = D * s, D * K
                if K >= 1:
                    sub = big.tile([P, 2, s, D, w], DT_BULK, tag=f"sA{idx}")
                    for c in range(2):
                        a_ap = bass.AP(
                            tensor=cxyf.tensor,
                            offset=cxyf.offset + (c * NPTS + o * D) * e,
                            ap=[pap, [D * e, s], [e, D], [0, w]])
                        b_ap = bass.AP(
                            tensor=c2f.tensor,
                            offset=c2f.offset + (c * DBL + dbl_off[o] + D) * e2,
                            ap=[pap2, [D * e2, s], [0, D], [e2, w]])
                        nc.vector.tensor_tensor(out=sub[:, c], in0=a_ap,
                                                in1=b_ap,
                                                op=mybir.AluOpType.subtract)
                    subsA[idx] = sub
                if half:
                    subh = big.tile([P, 2, half, D, D], DT_BULK, tag=f"sB{idx}")
                    for c in range(2):
                        a_ap = bass.AP(
                            tensor=cxyf.tensor,
                            offset=cxyf.offset + (c * NPTS + o * D) * e,
                            ap=[pap, [D * e, half], [e, D], [0, D]])
                        b_ap = bass.AP(
                            tensor=cxyf.tensor,
                            offset=cxyf.offset + (c * NPTS + (o + half) * D) * e,
                            ap=[pap, [D * e, half], [0, D], [e, D]])
                        nc.vector.tensor_tensor(out=subh[:, c], in0=a_ap,
                                                in1=b_ap,
                                                op=mybir.AluOpType.subtract)
                    subsB[idx] = subh

            def square_pair(sub, F, tagp):
                sq = {}
                subf = sub[:].rearrange("p c a b q -> p (c a b q)")
                es = subf.ap[-1][0]
                for c, nm in ((0, "x"), (1, "y")):
                    tsq = big.tile([P, F], DT_BULK, tag=f"{tagp}{nm}")
                    src_ap = bass.AP(tensor=subf.tensor,
                                     offset=subf.offset + c * F * es,
                                     ap=[subf.ap[0], [es, F]])
                    nc.scalar.activation(
                        out=tsq[:], in_=src_ap,
                        func=mybir.ActivationFunctionType.Square)
                    sq[nm] = tsq
                return sq

            sqsA, sqsB = {}, {}
            for idx, (o, s, K, half) in enumerate(rects):
                if K >= 1:
                    sqsA[idx] = square_pair(subsA[idx], s * D * D * K, f"qA{idx}")
                if half:
                    sqsB[idx] = square_pair(subsB[idx], half * D * D, f"qB{idx}")

            for idx, (o, s, K, half) in enumerate(rects):
                poff = poffs[idx]
                m, w = D * s, D * K
                if K >= 1:
                    F = s * D * w
                    d2 = big.tile([P, F], DT_BULK, tag=f"dA{idx}")
                    add_eng = nc.gpsimd if F <= GPSIMD_ADD_MAX_F else nc.vector
                    add_eng.tensor_tensor(out=d2[:], in0=sqsA[idx]["x"][:],
                                          in1=sqsA[idx]["y"][:],
                                          op=mybir.AluOpType.add)
                    # min over dj; scatter-write r1 in (i, k, di) order
                    ed = d2[:].ap[-1][0]
                    d2v = bass.AP(tensor=d2.tensor, offset=d2[:].offset,
                                  ap=[d2[:].ap[0], [w * ed, m],
                                      [D * ed, K], [ed, D]])
                    r1 = big.tile([P, s, K, D], DT_BULK, tag=f"rA{idx}")
                    r1f = r1[:].rearrange("p a b c -> p (a b c)")
                    e1 = r1f.ap[-1][0]
                    r1scat = bass.AP(tensor=r1f.tensor, offset=r1f.offset,
                                     ap=[r1f.ap[0], [K * D * e1, s],
                                         [e1, D], [D * e1, K]])
                    nc.vector.tensor_reduce(out=r1scat, in_=d2v,
                                            axis=mybir.AxisListType.X,
                                            op=mybir.AluOpType.min)
                    pmin = pdist[:, poff : poff + s * K].rearrange(
                        "p (a b) -> p a b", b=K)
                    nc.vector.tensor_reduce(out=pmin, in_=r1[:],
                                            axis=mybir.AxisListType.X,
                                            op=mybir.AluOpType.min)
                if half:
                    Fh = half * D * D
                    dh = big.tile([P, Fh], DT_BULK, tag=f"dB{idx}")
                    nc.vector.tensor_tensor(out=dh[:], in0=sqsB[idx]["x"][:],
                                            in1=sqsB[idx]["y"][:],
                                            op=mybir.AluOpType.add)
                    r1h = big.tile([P, half, D], DT_BULK, tag=f"rB{idx}")
                    nc.vector.tensor_reduce(
                        out=r1h[:],
                        in_=dh[:].rearrange("p (a dj) -> p a dj", dj=D),
                        axis=mybir.AxisListType.X, op=mybir.AluOpType.min)
                    ph = pdist[:, poff + s * K : poff + s * K + half]
                    nc.vector.tensor_reduce(out=ph, in_=r1h[:],
                                            axis=mybir.AxisListType.X,
                                            op=mybir.AluOpType.min)

            # ---- stage C: penalties on the packed half-pair list ----
            dist = small.tile([P, PP], F32, tag="dist")
            nc.scalar.activation(out=dist[:], in_=pdist[:],
                                 func=mybir.ActivationFunctionType.Sqrt)
            rr = small.tile([P, PP], F32, tag="rr")
            nc.vector.tensor_mul(rr[:], dist[:], prc)
            pen = small.tile([P, PP], F32, tag="pen")
            nc.scalar.activation(out=pen[:], in_=rr[:],
                                 func=mybir.ActivationFunctionType.Relu,
                                 bias=ones[:], scale=-1.0)

            # ---- j-sums: loss32(i) = row sums + partner sums ----
            loss32 = singles.tile([P, B], F32)
            penf = pen[:]
            epn = penf.ap[-1][0]
            for idx, (o, s, K, half) in enumerate(rects):
                poff = poffs[idx]
                if K >= 1:
                    pA = pen[:, poff : poff + s * K]
                    t1 = small.tile([P, s], F32, tag=f"t1{idx}")
                    nc.vector.tensor_reduce(
                        out=t1[:],
                        in_=pA.rearrange("p (a b) -> p a b", b=K),
                        axis=mybir.AxisListType.X, op=mybir.AluOpType.add)
                    # partner sums via a doubled copy and a sheared read:
                    # t2(i) = sum_k penA[(i+s-k) mod s, k]
                    pen2 = small.tile([P, 2, s * K], F32, tag=f"p2{idx}")
                    nc.vector.tensor_copy(
                        out=pen2[:],
                        in_=bass.AP(tensor=penf.tensor,
                                    offset=penf.offset + poff * epn,
                                    ap=[penf.ap[0], [0, 2], [epn, s * K]]))
                    p2f = pen2[:].rearrange("p c a -> p (c a)")
                    ep = p2f.ap[-1][0]
                    t2v = bass.AP(tensor=p2f.tensor,
                                  offset=p2f.offset + (s - 1) * K * ep,
                                  ap=[p2f.ap[0], [K * ep, s],
                                      [(1 - K) * ep, K]])
                    t2 = small.tile([P, s], F32, tag=f"t2{idx}")
                    nc.vector.tensor_reduce(out=t2[:], in_=t2v,
                                            axis=mybir.AxisListType.X,
                                            op=mybir.AluOpType.add)
                    nc.vector.tensor_add(loss32[:, o : o + s], t1[:], t2[:])
                else:
                    nc.vector.memset(loss32[:, o : o + s], 0.0)
                if half:
                    pB = pen[:, poff + s * K : poff + s * K + half]
                    nc.vector.tensor_add(loss32[:, o : o + half],
                                         loss32[:, o : o + half], pB)
                    nc.vector.tensor_add(loss32[:, o + half : o + s],
                                         loss32[:, o + half : o + s], pB)

            # ---- stage D ----
            ps = psum.tile([NL, B], F32)
            nc.tensor.matmul(ps[:], wt2[:], loss32[:], start=True, stop=True)
            lout = small.tile([NL, B], F32, tag="lout")
            nc.vector.tensor_mul(lout[:], ps[:], movt[:])
            nc.sync.dma_start(out=out[:], in_=lout[:])

    return nc


def _prepare(inputs):
    x = np.ascontiguousarray(inputs["x"], dtype=np.float32)
    extent = np.asarray(inputs["extent"], dtype=np.float32)
    wfa = np.asarray(inputs["world_from_agent"], dtype=np.float32)
    speed = np.asarray(inputs["curr_speed"], dtype=np.float32)
    scene = np.asarray(inputs["scene_index"])

    R = wfa[:, :2, :2]
    tr = wfa[:, :2, 2]
    yaw_off = np.arctan2(R[:, 1, 0], R[:, 0, 0]).astype(np.float32)
    agt_rad = extent[:, 1] / 2.0
    cent_min = -(extent[:, 0] / 2.0) + agt_rad
    cent_max = (extent[:, 0] / 2.0) - agt_rad
    lin = np.linspace(0.0, 1.0, D, dtype=np.float32)
    cent_x = (cent_min[:, None] + (cent_max - cent_min)[:, None] * lin).astype(
        np.float32)
    pd = (agt_rad[:, None] + agt_rad[None, :] + BUFFER_DIST).astype(np.float32)
    moving = (np.abs(speed) > SPEED_TH)

    # contiguous scene blocks (scene_index is sorted)
    _, starts, counts = np.unique(scene, return_index=True, return_counts=True)
    scenes = [(int(o), int(s)) for o, s in zip(starts, counts)]
    assert sum(s for _, s in scenes) == B
    for o, s in scenes:
        assert (scene[o : o + s] == scene[o]).all()

    pairs_i = []
    pairs_j = []
    for (o, s, K, half) in _rects(scenes):
        for i in range(s):
            for k in range(1, K + 1):
                pairs_i.append(o + i)
                pairs_j.append(o + (i + k) % s)
        for i in range(half):
            pairs_i.append(o + i)
            pairs_j.append(o + i + half)
    pairs_i = np.array(pairs_i)
    pairs_j = np.array(pairs_j)
    PP = len(pairs_i)
    inv_pd = (1.0 / pd[pairs_i, pairs_j]).astype(np.float32)

    twopi = 2.0 * np.pi
    geo = np.concatenate([
        R[:, 0, 0], R[:, 1, 0],          # gA
        R[:, 0, 1], R[:, 1, 1],          # gB
        tr[:, 0], tr[:, 1],              # gT
        2.0 + yaw_off / twopi, 2.25 + yaw_off / twopi,  # shifts2
    ]).astype(np.float32)

    w = DECAY_RATE ** np.arange(T, dtype=np.float32)
    w = w / w.sum()
    wmt = np.zeros((P, NL), dtype=np.float32)
    for nl in range(NL):
        wmt[nl * T : (nl + 1) * T, nl] = w / B

    # packed inputs: per-partition x data + replicated consts + wmt
    XWB = _xin_width_b(PP)
    mvr2 = np.tile(moving.astype(np.float32), NL)
    constA = geo
    xinb_row = np.empty((P, XWB), dtype=np.float32)
    xinb_row[:, XO_CENT : XO_PRC + PP] = np.concatenate(
        [cent_x.reshape(-1), mvr2, inv_pd])[None, :]
    xinb_row[:, XO_PRC + PP :] = wmt
    in_maps = []
    for c in range(NCORES):
        xs = x[:, c * NL : (c + 1) * NL, :, :]          # (B, NL, T, 6)
        xs = xs[..., [0, 1, 3]]                          # (B, NL, T, 3)
        xdat = xs.transpose(1, 2, 3, 0).reshape(P, 3 * B)
        xina = np.empty((P, XWA), dtype=np.float32)
        xina[:, 0 : 3 * B] = xdat
        xina[:, XO_GEO:] = constA[None, :]
        in_maps.append({"xina": xina, "xinb": xinb_row})

    return scenes, PP, in_maps, moving


_CACHE = {}


def _get_nc(scenes, PP):
    key = (tuple(scenes), PP)
    if key not in _CACHE:
        _CACHE[key] = _build_nc(scenes, PP)
    return _CACHE[key]


def _run(inputs, trace=False):
    scenes, PP, in_maps, moving = _prepare(inputs)
    nc = _get_nc(scenes, PP)
    res = run_bass_kernel_spmd(nc, in_maps, core_ids=list(range(NCORES)),
                               trace=trace)
    outf = np.zeros((B, N), dtype=np.float32)
    for c in range(NCORES):
        lc = res.results[c]["loss"]                      # (NL, B)
        for nl in range(NL):
            outf[:, c * NL + nl] = lc[nl]
    return outf, res


def kernel(**inputs):
    outf, _ = _run(inputs, trace=False)
    return outf


def _ensure_ntff_hook():
    """Register the axon NTFF profile hook if the container's antenv lacks it."""
    try:
        from antenv.axon_hooks import get_axon_ntff_profile_hook  # noqa: F401
        return
    except ImportError:
        pass
    import types

    if "/root/.axon_site" not in sys.path:
        sys.path.insert(0, "/root/.axon_site")
    from trn_agent_boot.trn_boot import _ntff_profile_via_ctypes

    hook = _ntff_profile_via_ctypes("/opt/axon/libaxon_pjrt.so")
    mod = types.ModuleType("antenv.axon_hooks")
    mod.get_axon_ntff_profile_hook = lambda: hook
    mod.set_axon_ntff_profile_hook = lambda h: None
    sys.modules["antenv.axon_hooks"] = mod


def run_traced(inputs):
    """Correctness output + profiled exec time (ns) via NTFF trace."""
    _ensure_ntff_hook()
    outf, res = _run(inputs, trace=True)
    return outf, res.exec_time_ns


# revision 60
# speedup vs baseline: 1.1414x; 1.0045x over previous
"""AgentCollisionLoss Trainium2 kernel.

Full inputs -> full output. Shards the N (sample) dim across 8 NeuronCores
(2 samples per core), computes the pairwise agent-collision loss on device,
and gathers the per-core (NL, B) losses into the full (B, N) output.

Device layout (per core):
  partition p = n_local*T + t            (104 rows)
  Stage A: world-frame disk centroids CXY [P, 2*B*D] from x + per-agent consts
  Stage B: per scene block, outer-difference over the packed disk-point list,
           squares on ACT, add + two-stage min-reduce over (dj, di) on DVE
  Stage C: sqrt, penalty = relu(1 - dist/pd) on the packed pair list
  Stage D: time-decay-weighted sum over t via a [P,2]^T @ [P,32] matmul,
           moving-mask, DMA out [2, 32]

All broadcast constants ride in the packed input tensors (replicated per
partition on the host): xina carries the stage-A-critical columns so compute
starts as soon as it lands; xinb carries the stage-C constants.
"""

import os
import sys

import numpy as np

for _p in ("/opt/trn_rl_repo", "/root/.axon_site/_ro/trn_rl_repo"):
    if os.path.isdir(_p) and _p not in sys.path:
        sys.path.insert(0, _p)

import bass_rust
import concourse.bass as bass
import concourse.mybir as mybir
import concourse.tile as tile
from concourse.bass_utils import run_bass_kernel_spmd
from concourse.vector_clock import ScopedClock


def _split_drain_and_barrier(self, tick_clock, wait_clock):
    """Kernel-tail drain, one semaphore per drain instruction.

    The walrus build in this container rejects instructions carrying more
    than one embedded sync wait ("Too many sync wait commands"). Tile's
    stock tail emits a single drain waiting on the full global clock, so
    split it: one drain per nonzero proc tick. add_sem_waits elides waits
    the engine has already observed, so each drain carries exactly one.
    """
    gc = list(tick_clock.global_clock)
    engs = [self.nc.sync, self.nc.vector, self.nc.scalar, self.nc.gpsimd,
            self.nc.tensor]
    nd = 0
    for idx, tick in enumerate(gc):
        if tick <= 0:
            continue
        v = [0] * len(gc)
        v[idx] = tick
        d = engs[nd % len(engs)].drain()
        nd += 1
        wait_clock.add_sem_waits(
            d.ins, ScopedClock({None: bass_rust.VectorClock(v)})
        )
    self.nc.all_engine_barrier()
    assert self.sems is not None
    popped = self.nc._tile_sem_poison_stack.pop()
    assert popped is self._sem_poison
    self.nc.clear_and_free_semaphores(list(self.sems.allocated().values()))
    self.nc.all_engine_barrier()


tile.TileContext._drain_and_barrier = _split_drain_and_barrier

B, N, T, D = 32, 16, 52, 5
NCORES = 8
NL = N // NCORES          # samples per core
P = NL * T                # partition rows per core
BUFFER_DIST = 0.2
DECAY_RATE = 0.9
SPEED_TH = 0.5
FMAX = 4000               # max free elems per big-stage chunk

F32 = mybir.dt.float32
F16 = mybir.dt.float16
PI = float(np.pi)

# bulk dtype for squared distances (precision analysed: d2 < 43000 < f16 max,
# and only d2 <= pd^2 ~ 16 matters, where f16 ulp ~ 0.008-0.016)
DT_BULK = F16
# gpsimd measured rates: 2-input fp16 add ~2.05 ns/elem, but broadcast-AP
# f32 subs ~3-3.6 ns/elem (worse than leaving them on DVE). So only the
# d2-adds of the non-largest chunks go to gpsimd.
SUB_ON_GPSIMD = ()
GPSIMD_ADD_MAX_F = 950


def _rects(scenes):
    """Circulant half-pair rects per scene, largest first.

    Each unordered same-scene pair {i, j} is covered exactly once:
    rect A: (i, k) for i in [0,s), k in [1,K], j = (i+k) mod s, K=(s-1)//2
    rect B (even s): (i, s/2) for i in [0, s/2), j = i + s/2
    Returns [(o, s, K, half)] with half = s//2 if s even else 0.
    """
    out = []
    for (o, s) in scenes:
        K = (s - 1) // 2
        half = s // 2 if s % 2 == 0 else 0
        out.append((o, s, K, half))
    out.sort(key=lambda r: -(r[1] * r[2] + r[3]))
    return out


# xinA column layout (stage-A-critical): x(3B) | geo(8B)
XO_GEO = 3 * B
XWA = XO_GEO + 8 * B
# xinB column layout: cent(B*D) | mvr(NL*B) | prc(PP) | wmt(NL)
XO_CENT = 0
XO_MVR = XO_CENT + B * D
XO_PRC = XO_MVR + NL * B


def _xin_width_b(PP):
    return XO_PRC + PP + NL


def _build_nc(scenes, PP):
    """Build the SPMD Bass program. `scenes` = [(offset, size)], PP = sum s^2."""
    nc = bass.Bass()

    XWB = _xin_width_b(PP)
    xina = nc.dram_tensor("xina", [P, XWA], F32, kind="ExternalInput")
    xinb = nc.dram_tensor("xinb", [P, XWB], F32, kind="ExternalInput")
    out = nc.dram_tensor("loss", [NL, B], F32, kind="ExternalOutput")

    rects = _rects(scenes)

    with tile.TileContext(nc) as tc:
        with (
            tc.tile_pool(name="singles", bufs=1) as singles,
            tc.tile_pool(name="small", bufs=1) as small,
            tc.tile_pool(name="big", bufs=1) as big,
            tc.tile_pool(name="psum", bufs=1, space="PSUM") as psum,
        ):
            # ---- loads (stage-A-critical part first) ----
            xta = singles.tile([P, XWA], F32)
            nc.sync.dma_start(out=xta[:], in_=xina[:])
            xtb = singles.tile([P, XWB], F32)
            nc.sync.dma_start(out=xtb[:], in_=xinb[:])
            ones = singles.tile([P, 1], F32)
            nc.vector.memset(ones[:], 1.0)

            # Pre-touch the DMA'd tiles on DVE: each copy carries one
            # DMA-queue sem wait, so later compute ops joining DMA data with
            # engine-produced data need at most one new wait (this walrus
            # rejects instructions with more than one embedded sync wait).
            tch = singles.tile([P, 1], F32, tag="tch0")
            nc.vector.tensor_copy(out=tch[:], in_=xta[:, 0:1])
            tchb = singles.tile([P, 1], F32, tag="tchb")
            nc.vector.tensor_copy(out=tchb[:], in_=xtb[:, 0:1])
            # matmul weights via DVE so the PE matmul's deps are DVE-only
            wt2 = singles.tile([P, NL], F32)
            nc.vector.tensor_copy(out=wt2[:], in_=xtb[:, XO_PRC + PP : XO_PRC + PP + NL])

            gA = xta[:, XO_GEO + 0 * B : XO_GEO + 2 * B]
            gB = xta[:, XO_GEO + 2 * B : XO_GEO + 4 * B]
            gT = xta[:, XO_GEO + 4 * B : XO_GEO + 6 * B]
            shifts2 = xta[:, XO_GEO + 6 * B : XO_GEO + 8 * B]
            x0 = xta[:, 0:B]
            x1 = xta[:, B : 2 * B]
            yw = xta[:, 2 * B : 3 * B]
            cxc = xtb[:, XO_CENT : XO_CENT + B * D]
            movt = xtb[0:NL, XO_MVR : XO_MVR + B]   # replicated const rows
            prc = xtb[:, XO_PRC : XO_PRC + PP]

            def rep2(apx, w):
                """view [P, 2, w] reading apx's first w elems twice"""
                return bass.AP(tensor=apx.tensor, offset=apx.offset,
                               ap=[apx.ap[0], [0, 2], [1, w]])

            # ---- stage A ----
            # u = yaw/2pi + (shift + yoff/2pi)   (shift 2.0 -> sin, 2.25 -> cos)
            u2 = small.tile([P, 2, B], F32)
            nc.vector.scalar_tensor_tensor(
                out=u2[:], in0=rep2(yw, B), scalar=1.0 / (2.0 * PI),
                in1=shifts2.rearrange("p (c i) -> p c i", c=2),
                op0=mybir.AluOpType.mult, op1=mybir.AluOpType.add)
            # round-to-nearest-even via the 1.5*2^23 magic constant
            MAGIC = 12582912.0
            kf = small.tile([P, 2, B], F32)
            nc.vector.tensor_scalar(
                out=kf[:], in0=u2[:], scalar1=MAGIC, scalar2=MAGIC,
                op0=mybir.AluOpType.add, op1=mybir.AluOpType.subtract)
            fr = small.tile([P, 2, B], F32)
            nc.vector.tensor_sub(fr[:], u2[:], kf[:])
            # sincos[:, 0:32] = sin(yawg), [:, 32:64] = cos(yawg)
            sincos = small.tile([P, 2 * B], F32)
            nc.scalar.activation(out=sincos[:].rearrange("p (c i) -> p c i", c=2),
                                 in_=fr[:],
                                 func=mybir.ActivationFunctionType.Sin,
                                 bias=0.0, scale=2.0 * PI)

            # pos_g for both coords: pg[p, c, i], c=0 -> x, 1 -> y
            # m12[p, xsel, c, i] = x_xsel * g_{xsel,c}  in one multiply
            m12 = small.tile([P, 2, 2, B], F32)
            xx = bass.AP(tensor=xta.tensor, offset=x0.offset,
                         ap=[x0.ap[0], [B, 2], [0, 2], [1, B]])
            gAB = bass.AP(tensor=xta.tensor, offset=gA.offset,
                          ap=[gA.ap[0], [2 * B, 2], [B, 2], [1, B]])
            nc.vector.tensor_mul(m12[:], xx, gAB)
            pg = small.tile([P, 2, B], F32)
            nc.vector.tensor_add(pg[:], m12[:, 0], m12[:, 1])
            nc.vector.tensor_add(pg[:], pg[:],
                                 gT.rearrange("p (c i) -> p c i", c=2))

            # CXY[p, c, i, di] = cent_x(i,di) * cs(c,i) + pg(c,i)
            # c=0 uses cos, c=1 uses sin (x = cx*cos + pgx, y = cx*sin + pgy)
            cxy = singles.tile([P, 2, B, D], F32)
            cs_sel = bass.AP(tensor=sincos.tensor, offset=sincos[:].offset + B,
                             ap=[sincos[:].ap[0], [-B, 2], [1, B], [0, D]])
            cx_rep = bass.AP(tensor=xtb.tensor, offset=cxc.offset,
                             ap=[cxc.ap[0], [0, 2], [D, B], [1, D]])
            pg_bc = bass.AP(tensor=pg.tensor, offset=pg[:].offset,
                            ap=[pg[:].ap[0], [B, 2], [1, B], [0, D]])
            nc.vector.tensor_mul(cxy[:], cx_rep, cs_sel)
            nc.vector.tensor_add(cxy[:], cxy[:], pg_bc)

            cxyf = cxy[:].rearrange("p c i d -> p (c i d)")
            pap = cxyf.ap[0]
            e = cxyf.ap[-1][0]

            # ---- stage B: circulant half-pair rects ----
            pdist = singles.tile([P, PP], F32)
            NPTS = B * D

            # doubled per-scene point lists so the wrap in j = (i+k) mod s
            # becomes a plain linear read: cxy2 block for scene (o,s) holds
            # its 5s points twice, per coord
            DBL = 2 * NPTS
            cxy2 = singles.tile([P, 2, DBL], F32)
            c2f = cxy2[:].rearrange("p c d -> p (c d)")
            pap2 = c2f.ap[0]
            e2 = c2f.ap[-1][0]
            dbl_off = {}
            do_ = 0
            for (o, s) in scenes:
                dbl_off[o] = do_
                in_ap = bass.AP(tensor=cxyf.tensor,
                                offset=cxyf.offset + o * D * e,
                                ap=[pap, [NPTS * e, 2], [0, 2], [e, D * s]])
                out_ap = bass.AP(tensor=c2f.tensor,
                                 offset=c2f.offset + do_ * e2,
                                 ap=[pap2, [DBL * e2, 2], [D * s * e2, 2],
                                     [e2, D * s]])
                nc.vector.tensor_copy(out=out_ap, in_=in_ap)
                do_ += 2 * D * s

            poffs = []
            po = 0
            for (o, s, K, half) in rects:
                poffs.append(po)
                po += s * K + half
            assert po == PP

            subsA, subsB = {}, {}
            for idx, (o, s, K, half) in enumerate(rects):
                m, w = D * s, D * K
                if K >= 1:
                    sub = big.tile([P, 2, s, D, w], DT_BULK, tag=f"sA{idx}")
                    for c in range(2):
                        a_ap = bass.AP(
                            tensor=cxyf.tensor,
                            offset=cxyf.offset + (c * NPTS + o * D) * e,
                            ap=[pap, [D * e, s], [e, D], [0, w]])
                        b_ap = bass.AP(
                            tensor=c2f.tensor,
                            offset=c2f.offset + (c * DBL + dbl_off[o] + D) * e2,
                            ap=[pap2, [D * e2, s], [0, D], [e2, w]])
                        nc.vector.tensor_tensor(out=sub[:, c], in0=a_ap,
                                                in1=b_ap,
                                                op=mybir.AluOpType.subtract)
                    subsA[idx] = sub
                if half:
                    subh = big.tile([P, 2, half, D, D], DT_BULK, tag=f"sB{idx}")
                    for c in range(2):
                        a_ap = bass.AP(
                            tensor=cxyf.tensor,
                            offset=cxyf.offset + (c * NPTS + o * D) * e,
                            ap=[pap, [D * e, half], [e, D], [0, D]])
                        b_ap = bass.AP(
                            tensor=cxyf.tensor,
                            offset=cxyf.offset + (c * NPTS + (o + half) * D) * e,
                            ap=[pap, [D * e, half], [0, D], [e, D]])
                        nc.vector.tensor_tensor(out=subh[:, c], in0=a_ap,
                                                in1=b_ap,
                                                op=mybir.AluOpType.subtract)
                    subsB[idx] = subh

            def square_pair(sub, F, tagp):
                sq = {}
                subf = sub[:].rearrange("p c a b q -> p (c a b q)")
                es = subf.ap[-1][0]
                for c, nm in ((0, "x"), (1, "y")):
                    tsq = big.tile([P, F], DT_BULK, tag=f"{tagp}{nm}")
                    src_ap = bass.AP(tensor=subf.tensor,
                                     offset=subf.offset + c * F * es,
                                     ap=[subf.ap[0], [es, F]])
                    nc.scalar.activation(
                        out=tsq[:], in_=src_ap,
                        func=mybir.ActivationFunctionType.Square)
                    sq[nm] = tsq
                return sq

            sqsA, sqsB = {}, {}
            for idx, (o, s, K, half) in enumerate(rects):
                if K >= 1:
                    sqsA[idx] = square_pair(subsA[idx], s * D * D * K, f"qA{idx}")
                if half:
                    sqsB[idx] = square_pair(subsB[idx], half * D * D, f"qB{idx}")

            for idx, (o, s, K, half) in enumerate(rects):
                poff = poffs[idx]
                m, w = D * s, D * K
                if K >= 1:
                    F = s * D * w
                    d2 = big.tile([P, F], DT_BULK, tag=f"dA{idx}")
                    add_eng = nc.gpsimd if F <= GPSIMD_ADD_MAX_F else nc.vector
                    add_eng.tensor_tensor(out=d2[:], in0=sqsA[idx]["x"][:],
                                          in1=sqsA[idx]["y"][:],
                                          op=mybir.AluOpType.add)
                    # min over dj; scatter-write r1 in (i, k, di) order
                    ed = d2[:].ap[-1][0]
                    d2v = bass.AP(tensor=d2.tensor, offset=d2[:].offset,
                                  ap=[d2[:].ap[0], [w * ed, m],
                                      [D * ed, K], [ed, D]])
                    r1 = big.tile([P, s, K, D], DT_BULK, tag=f"rA{idx}")
                    r1f = r1[:].rearrange("p a b c -> p (a b c)")
                    e1 = r1f.ap[-1][0]
                    r1scat = bass.AP(tensor=r1f.tensor, offset=r1f.offset,
                                     ap=[r1f.ap[0], [K * D * e1, s],
                                         [e1, D], [D * e1, K]])
                    nc.vector.tensor_reduce(out=r1scat, in_=d2v,
                                            axis=mybir.AxisListType.X,
                                            op=mybir.AluOpType.min)
                    pmin = pdist[:, poff : poff + s * K].rearrange(
                        "p (a b) -> p a b", b=K)
                    nc.vector.tensor_reduce(out=pmin, in_=r1[:],
                                            axis=mybir.AxisListType.X,
                                            op=mybir.AluOpType.min)
                if half:
                    Fh = half * D * D
                    dh = big.tile([P, Fh], DT_BULK, tag=f"dB{idx}")
                    nc.vector.tensor_tensor(out=dh[:], in0=sqsB[idx]["x"][:],
                                            in1=sqsB[idx]["y"][:],
                                            op=mybir.AluOpType.add)
                    r1h = big.tile([P, half, D], DT_BULK, tag=f"rB{idx}")
                    nc.vector.tensor_reduce(
                        out=r1h[:],
                        in_=dh[:].rearrange("p (a dj) -> p a dj", dj=D),
                        axis=mybir.AxisListType.X, op=mybir.AluOpType.min)
                    ph = pdist[:, poff + s * K : poff + s * K + half]
                    nc.vector.tensor_reduce(out=ph, in_=r1h[:],
                                            axis=mybir.AxisListType.X,
                                            op=mybir.AluOpType.min)

            # ---- per-rect stage C + j-sums, pipelined behind the mins ----
            # (squares were all emitted above, so the sqrt-table load happens
            # once; per-rect penalties use DVE tensor_scalar ops)
            loss32 = singles.tile([P, B], F32)
            for idx, (o, s, K, half) in enumerate(rects):
                poff = poffs[idx]
                cnt = s * K + half
                if cnt == 0:
                    nc.vector.memset(loss32[:, o : o + s], 0.0)
                    continue
                dist = small.tile([P, cnt], F32, tag=f"ds{idx}")
                nc.scalar.activation(out=dist[:],
                                     in_=pdist[:, poff : poff + cnt],
                                     func=mybir.ActivationFunctionType.Sqrt)
                rr = small.tile([P, cnt], F32, tag=f"rr{idx}")
                nc.vector.tensor_mul(rr[:], dist[:],
                                     prc[:, poff : poff + cnt])
                # pen = max(1 - rr, 0) on DVE
                pen = small.tile([P, cnt], F32, tag=f"pn{idx}")
                nc.vector.tensor_scalar(
                    out=pen[:], in0=rr[:], scalar1=1.0, scalar2=-1.0,
                    op0=mybir.AluOpType.subtract, op1=mybir.AluOpType.mult)
                nc.vector.tensor_scalar_max(out=pen[:], in0=pen[:],
                                            scalar1=0.0)
                penf = pen[:]
                epn = penf.ap[-1][0]
                if K >= 1:
                    t1 = small.tile([P, s], F32, tag=f"t1{idx}")
                    nc.vector.tensor_reduce(
                        out=t1[:],
                        in_=pen[:, 0 : s * K].rearrange(
                            "p (a b) -> p a b", b=K),
                        axis=mybir.AxisListType.X, op=mybir.AluOpType.add)
                    # partner sums via a doubled copy and a sheared read:
                    # t2(i) = sum_k penA[(i+s-k) mod s, k]
                    pen2 = small.tile([P, 2, s * K], F32, tag=f"p2{idx}")
                    nc.vector.tensor_copy(
                        out=pen2[:],
                        in_=bass.AP(tensor=penf.tensor, offset=penf.offset,
                                    ap=[penf.ap[0], [0, 2], [epn, s * K]]))
                    p2f = pen2[:].rearrange("p c a -> p (c a)")
                    ep = p2f.ap[-1][0]
                    t2v = bass.AP(tensor=p2f.tensor,
                                  offset=p2f.offset + (s - 1) * K * ep,
                                  ap=[p2f.ap[0], [K * ep, s],
                                      [(1 - K) * ep, K]])
                    t2 = small.tile([P, s], F32, tag=f"t2{idx}")
                    nc.vector.tensor_reduce(out=t2[:], in_=t2v,
                                            axis=mybir.AxisListType.X,
                                            op=mybir.AluOpType.add)
                    nc.vector.tensor_add(loss32[:, o : o + s], t1[:], t2[:])
                else:
                    nc.vector.memset(loss32[:, o : o + s], 0.0)
                if half:
                    pB = pen[:, s * K : s * K + half]
                    nc.vector.tensor_add(loss32[:, o : o + half],
                                         loss32[:, o : o + half], pB)
                    nc.vector.tensor_add(loss32[:, o + half : o + s],
                                         loss32[:, o + half : o + s], pB)

            # ---- stage D ----
            ps = psum.tile([NL, B], F32)
            nc.tensor.matmul(ps[:], wt2[:], loss32[:], start=True, stop=True)
            lout = small.tile([NL, B], F32, tag="lout")
            nc.vector.tensor_mul(lout[:], ps[:], movt[:])
            nc.sync.dma_start(out=out[:], in_=lout[:])

    return nc


def _prepare(inputs):
    x = np.ascontiguousarray(inputs["x"], dtype=np.float32)
    extent = np.asarray(inputs["extent"], dtype=np.float32)
    wfa = np.asarray(inputs["world_from_agent"], dtype=np.float32)
    speed = np.asarray(inputs["curr_speed"], dtype=np.float32)
    scene = np.asarray(inputs["scene_index"])

    R = wfa[:, :2, :2]
    tr = wfa[:, :2, 2]
    yaw_off = np.arctan2(R[:, 1, 0], R[:, 0, 0]).astype(np.float32)
    agt_rad = extent[:, 1] / 2.0
    cent_min = -(extent[:, 0] / 2.0) + agt_rad
    cent_max = (extent[:, 0] / 2.0) - agt_rad
    lin = np.linspace(0.0, 1.0, D, dtype=np.float32)
    cent_x = (cent_min[:, None] + (cent_max - cent_min)[:, None] * lin).astype(
        np.float32)
    pd = (agt_rad[:, None] + agt_rad[None, :] + BUFFER_DIST).astype(np.float32)
    moving = (np.abs(speed) > SPEED_TH)

    # contiguous scene blocks (scene_index is sorted)
    _, starts, counts = np.unique(scene, return_index=True, return_counts=True)
    scenes = [(int(o), int(s)) for o, s in zip(starts, counts)]
    assert sum(s for _, s in scenes) == B
    for o, s in scenes:
        assert (scene[o : o + s] == scene[o]).all()

    pairs_i = []
    pairs_j = []
    for (o, s, K, half) in _rects(scenes):
        for i in range(s):
            for k in range(1, K + 1):
                pairs_i.append(o + i)
                pairs_j.append(o + (i + k) % s)
        for i in range(half):
            pairs_i.append(o + i)
            pairs_j.append(o + i + half)
    pairs_i = np.array(pairs_i)
    pairs_j = np.array(pairs_j)
    PP = len(pairs_i)
    inv_pd = (1.0 / pd[pairs_i, pairs_j]).astype(np.float32)

    twopi = 2.0 * np.pi
    geo = np.concatenate([
        R[:, 0, 0], R[:, 1, 0],          # gA
        R[:, 0, 1], R[:, 1, 1],          # gB
        tr[:, 0], tr[:, 1],              # gT
        2.0 + yaw_off / twopi, 2.25 + yaw_off / twopi,  # shifts2
    ]).astype(np.float32)

    w = DECAY_RATE ** np.arange(T, dtype=np.float32)
    w = w / w.sum()
    wmt = np.zeros((P, NL), dtype=np.float32)
    for nl in range(NL):
        wmt[nl * T : (nl + 1) * T, nl] = w / B

    # packed inputs: per-partition x data + replicated consts + wmt
    XWB = _xin_width_b(PP)
    mvr2 = np.tile(moving.astype(np.float32), NL)
    constA = geo
    xinb_row = np.empty((P, XWB), dtype=np.float32)
    xinb_row[:, XO_CENT : XO_PRC + PP] = np.concatenate(
        [cent_x.reshape(-1), mvr2, inv_pd])[None, :]
    xinb_row[:, XO_PRC + PP :] = wmt
    in_maps = []
    for c in range(NCORES):
        xs = x[:, c * NL : (c + 1) * NL, :, :]          # (B, NL, T, 6)
        xs = xs[..., [0, 1, 3]]                          # (B, NL, T, 3)
        xdat = xs.transpose(1, 2, 3, 0).reshape(P, 3 * B)
        xina = np.empty((P, XWA), dtype=np.float32)
        xina[:, 0 : 3 * B] = xdat
        xina[:, XO_GEO:] = constA[None, :]
        in_maps.append({"xina": xina, "xinb": xinb_row})

    return scenes, PP, in_maps, moving


_CACHE = {}


def _get_nc(scenes, PP):
    key = (tuple(scenes), PP)
    if key not in _CACHE:
        _CACHE[key] = _build_nc(scenes, PP)
    return _CACHE[key]


def _run(inputs, trace=False):
    scenes, PP, in_maps, moving = _prepare(inputs)
    nc = _get_nc(scenes, PP)
    res = run_bass_kernel_spmd(nc, in_maps, core_ids=list(range(NCORES)),
                               trace=trace)
    outf = np.zeros((B, N), dtype=np.float32)
    for c in range(NCORES):
        lc = res.results[c]["loss"]                      # (NL, B)
        for nl in range(NL):
            outf[:, c * NL + nl] = lc[nl]
    return outf, res


def kernel(**inputs):
    outf, _ = _run(inputs, trace=False)
    return outf


def _ensure_ntff_hook():
    """Register the axon NTFF profile hook if the container's antenv lacks it."""
    try:
        from antenv.axon_hooks import get_axon_ntff_profile_hook  # noqa: F401
        return
    except ImportError:
        pass
    import types

    if "/root/.axon_site" not in sys.path:
        sys.path.insert(0, "/root/.axon_site")
    from trn_agent_boot.trn_boot import _ntff_profile_via_ctypes

    hook = _ntff_profile_via_ctypes("/opt/axon/libaxon_pjrt.so")
    mod = types.ModuleType("antenv.axon_hooks")
    mod.get_axon_ntff_profile_hook = lambda: hook
    mod.set_axon_ntff_profile_hook = lambda h: None
    sys.modules["antenv.axon_hooks"] = mod


def run_traced(inputs):
    """Correctness output + profiled exec time (ns) via NTFF trace."""
    _ensure_ntff_hook()
    outf, res = _run(inputs, trace=True)
    return outf, res.exec_time_ns
